# revision 1
# baseline (speedup 1.0000x reference)
# Bass/Trainium2 kernel for nn_ABELSpline (embedding_lookup via implicit one-hot matmul).
#
# Math: out[b,o] = sum_d sum_r B(256*x[b,d] + 3 - r) * table[d*259 + r, o]
#   where B is the cubic B-spline basis on [0,4). The gather/multiply/reduce of the
#   reference is exactly a dense matmul with W^T[r, b] = B(...) built on the fly:
#     B(y) = relu(2-u)^3/6 - (2/3)*relu(1-u)^3,  u = |y-2|
#   computed by two fused custom DVE ops (7/8 ALU stages), then TensorE contracts
#   W^T against the (bf16) tables with PSUM accumulation.
#
# Sharding: cores form a 2x4 grid: batch halves (512) x dim quarters (32 dims).
# All 4 local batch-chunks accumulate in PSUM concurrently so TensorE pipelines
# fully with the DVE W-build. A 4-core bf16 ReduceScatter per batch half sums
# partials; each core applies the anti-symmetric-exponential tail on its
# 128-batch slice; the host concatenates the 8 slices.

import numpy as np

BATCH = 1024
INPUT_DIM = 128
DENSITY = 259
OUTPUT_DIM = 64
NUM_EXPS = 4
INDIRECT_DIM = 2 * NUM_EXPS * OUTPUT_DIM  # 512
NCOLS = INDIRECT_DIM + OUTPUT_DIM  # 576
NCORES = 8
DGROUPS = 4                 # dim-parallel cores per batch half
BGROUPS = NCORES // DGROUPS  # 2 batch halves
DPC = INPUT_DIM // DGROUPS  # dims per core = 32
BL = BATCH // BGROUPS       # local batch = 512
MBL = BL // 128             # local batch chunks = 4
GDIMS = 4                   # dims per W build group
NGROUPS = DPC // GDIMS      # 8
MAIN_ROWS = 256
EDGE_ROWS = DENSITY - MAIN_ROWS  # 3
EP = DPC * EDGE_ROWS        # 96 edge partitions
SCALE = float(DENSITY - 3)  # 256
K1 = float((1.0 / 6.0) ** (1.0 / 3.0))
K2 = float((4.0 / 6.0) ** (1.0 / 3.0))

DEBUG_PARTIALS = False

_OPS = None
_NC = None


def _register_ops():
    global _OPS
    if _OPS is not None:
        return _OPS
    from concourse.dve_spec import Spec, Src0, Src1, C0, C1, C2, Bin, relu, sq, minn, lower
    from concourse.dve_uop import AluOp, DveOpSpec
    from concourse import dve_ops
    from concourse.dve_ops import DveOp

    def relu3_body():
        # relu(min(C0 - m, m + C1))^3 with m = Src0*C2
        m = Src0 * C2
        a = Bin(AluOp.SUBTRACT, C0, m)
        b = Bin(AluOp.ADD, m, C1)
        r = relu(minn(a, b))
        return r * sq(r)

    def ref_a(in0, in1, s0, s1, imm2):
        t = np.maximum(np.minimum(s0 - in0 * imm2, in0 * imm2 + s1), 0.0)
        return (t ** 3).astype(np.float32)

    def ref_b(in0, in1, s0, s1, imm2):
        t = np.maximum(np.minimum(s0 - in0 * imm2, in0 * imm2 + s1), 0.0)
        return (in1 - t ** 3).astype(np.float32)

    spec_a = Spec(body=relu3_body(), reference=ref_a)
    spec_b = Spec(body=Bin(AluOp.SUBTRACT, Src1, relu3_body()), reference=ref_b)

    ops = []
    for name, spec in (("SPLINE3A_ANT", spec_a), ("SPLINE3B_ANT", spec_b)):
        if name in dve_ops._SUB_OPCODE_FOR_NAME:
            ops.append(next(o for o in dve_ops.OPS if o.name == name))
            continue
        row = max(dve_ops._SUB_OPCODE_FOR_NAME.values()) + 1
        assert row < 0x20
        uops = lower(spec, ver="v3")
        rd1 = name.endswith("B_ANT")
        sha = DveOpSpec(name=name, opcode=row, uops=uops, rd1_en=rd1).sha("v3")
        op = DveOp(name, spec, subdim=False, uops_sha={"v3": sha})
        dve_ops.OPS.append(op)
        dve_ops._SUB_OPCODE_FOR_NAME[name] = row
        dve_ops.CUSTOM_DVE_SPECS[name] = spec
        ops.append(op)
    _OPS = tuple(ops)
    return _OPS


def _bias_consts():
    # [128, 16] f32 per-partition scalars.
    # cols 4h+{0..3}: main half h (r = 128h + p): K1*(r+1), K1*(3-r), K2*r, K2*(2-r)
    # cols 8..11: edge (r = 256 + p%3)
    b = np.zeros((128, 16), np.float32)
    p = np.arange(128, dtype=np.float64)
    for h in (0, 1):
        r = 128 * h + p
        b[:, 4 * h + 0] = K1 * (r + 1)
        b[:, 4 * h + 1] = K1 * (3 - r)
        b[:, 4 * h + 2] = K2 * r
        b[:, 4 * h + 3] = K2 * (2 - r)
    re = 256 + (p % 3)
    b[:, 8] = K1 * (re + 1)
    b[:, 9] = K1 * (3 - re)
    b[:, 10] = K2 * re
    b[:, 11] = K2 * (2 - re)
    return b


def _wpat_const():
    w = np.array([1.0 / (i + 1) ** 2 for i in range(NUM_EXPS)], np.float32)
    row = np.concatenate([w, -w])  # [8]
    return np.tile(row, (128, OUTPUT_DIM)).astype(np.float32)  # [128, 512]


def _build(skip=(), loop_reps=0, reps=1):
    global _NC
    if _NC is not None and not skip and not loop_reps and reps == 1:
        return _NC
    import contextlib
    import concourse.bass as bass
    import concourse.bacc as bacc
    import concourse.tile as tile
    import concourse.mybir as mybir

    OP_A, OP_B = _register_ops()
    f32 = mybir.dt.float32
    bf16 = mybir.dt.bfloat16

    nc = bacc.Bacc("TRN2", target_bir_lowering=False, debug=False, num_devices=NCORES)

    xt_d = nc.dram_tensor("xt", [DPC, BL], f32, kind="ExternalInput")
    xe_d = nc.dram_tensor("xe", [EP, BL], f32, kind="ExternalInput")
    # tmain pre-permuted on host: [128, (2*DPC)*NCOLS], chunk c=2*di+h at cols c*NCOLS
    tmain_d = nc.dram_tensor("tmain", [128, 2 * DPC * NCOLS], f32, kind="ExternalInput")
    tedge_d = nc.dram_tensor("tedge", [EP, NCOLS], f32, kind="ExternalInput")
    bias_d = nc.dram_tensor("bias", [128, 16], f32, kind="ExternalInput")
    wpat_d = nc.dram_tensor("wpat", [128, INDIRECT_DIM], f32, kind="ExternalInput")
    if DEBUG_PARTIALS:
        out_d = nc.dram_tensor("out", [BL, NCOLS], f32, kind="ExternalOutput")
    else:
        out_d = nc.dram_tensor("out", [128, OUTPUT_DIM], f32, kind="ExternalOutput")

    rgroups = [[g * DGROUPS + i for i in range(DGROUPS)] for g in range(BGROUPS)]

    with tile.TileContext(nc) as tc:
        with (
            tc.tile_pool(name="const", bufs=1) as cpool,
            tc.tile_pool(name="tbl", bufs=3) as tpool,
            tc.tile_pool(name="tble", bufs=1) as tepool,
            tc.tile_pool(name="xrep", bufs=2) as xrpool,
            tc.tile_pool(name="w1", bufs=2) as w1pool,
            tc.tile_pool(name="W", bufs=6) as wpool,
            tc.tile_pool(name="acc", bufs=4, space="PSUM") as psumpool,
            tc.tile_pool(name="part", bufs=2) as partpool,
            tc.tile_pool(name="fin", bufs=4) as finpool,
            tc.tile_pool(name="dram", bufs=1, space="DRAM") as dpool,
        ):
            # ---- constants ----
            bias_s = cpool.tile([128, 16], f32, tag="bias")
            nc.sync.dma_start(bias_s[:], bias_d[:])
            xe_s = cpool.tile([EP, BL], f32, tag="xe")
            nc.sync.dma_start(xe_s[:], xe_d[:])
            if not DEBUG_PARTIALS:
                wpat_s = cpool.tile([128, INDIRECT_DIM], f32, tag="wpat")
                nc.sync.dma_start(wpat_s[:], wpat_d[:])

            loop_cm = tc.For_i(0, loop_reps, 1) if loop_reps else contextlib.nullcontext()
            with loop_cm:
              for _rep in range(reps):
                  # interleaved input DMAs: per group g, x-broadcast + table quarter
                  xrs, tqs = [], []
                  GCH = 2 * GDIMS * NCOLS  # table cols per group chunk-set
                  for g in range(NGROUPS):
                      xr = xrpool.tile([128, GDIMS * BL], f32, tag="xrep", name=f"xr{g}")
                      src = bass.AP(xt_d, 0, [[0, 128], [1, GDIMS * BL]])
                      src.offset = g * GDIMS * BL
                      nc.sync.dma_start(xr[:], src)
                      xrs.append(xr)
                      tq = tpool.tile([128, GCH], bf16, tag="tbl", name=f"tq{g}")
                      nc.gpsimd.dma_start(tq[:], tmain_d[:, g * GCH:(g + 1) * GCH])
                      tqs.append(tq)
                  te_s = tepool.tile([EP, NCOLS], bf16, tag="Te")
                  nc.gpsimd.dma_start(te_s[:], tedge_d[:])

                  psums = None
                  if "pe" not in skip:
                      psums = [psumpool.tile([128, NCOLS], f32, tag="acc", name=f"ps{m}")
                               for m in range(MBL)]

                  for g in range(NGROUPS):
                      Wg = {}
                      if "dve" not in skip:
                          for h in (0, 1):
                              w1 = w1pool.tile([128, GDIMS * BL], bf16, tag="w1")
                              nc.vector._custom_dve(
                                  OP_A, out=w1[:], in0=xrs[g][:],
                                  s0=bias_s[:, 4 * h:4 * h + 1], s1=bias_s[:, 4 * h + 1:4 * h + 2],
                                  imm2=SCALE * K1)
                              W = wpool.tile([128, GDIMS * BL], bf16, tag="W", name=f"W{g}_{h}")
                              nc.vector._custom_dve(
                                  OP_B, out=W[:], in0=xrs[g][:], in1=w1[:],
                                  s0=bias_s[:, 4 * h + 2:4 * h + 3], s1=bias_s[:, 4 * h + 3:4 * h + 4],
                                  imm2=SCALE * K2)
                              Wg[h] = W
                      if "pe" not in skip and "dve" not in skip:
                          for m in range(MBL):
                              for h in (0, 1):
                                  for di in range(GDIMS):
                                      lhsT = Wg[h][:, di * BL + 128 * m:di * BL + 128 * m + 128]
                                      rhs = tqs[g][:, (2 * di + h) * NCOLS:(2 * di + h + 1) * NCOLS]
                                      st = g == 0 and h == 0 and di == 0
                                      nc.tensor.matmul(psums[m][:, 0:512], lhsT, rhs[:, 0:512],
                                                       start=st, stop=False)
                                      nc.tensor.matmul(psums[m][:, 512:NCOLS], lhsT, rhs[:, 512:NCOLS],
                                                       start=st, stop=False)

                  if "dve" not in skip:
                      w1e = w1pool.tile([EP, BL], bf16, tag="w1", name="w1e")
                      nc.vector._custom_dve(
                          OP_A, out=w1e[:], in0=xe_s[:],
                          s0=bias_s[:EP, 8:9], s1=bias_s[:EP, 9:10], imm2=SCALE * K1)
                      We = wpool.tile([EP, BL], bf16, tag="We", name="We")
                      nc.vector._custom_dve(
                          OP_B, out=We[:], in0=xe_s[:], in1=w1e[:],
                          s0=bias_s[:EP, 10:11], s1=bias_s[:EP, 11:12], imm2=SCALE * K2)

                  pdt = f32 if DEBUG_PARTIALS else bf16
                  pin = dpool.tile([BL, NCOLS], pdt, tag="pin")
                  if "pe" not in skip and "dve" not in skip:
                      for m in range(MBL):
                          lhsT = We[:, 128 * m:128 * m + 128]
                          nc.tensor.matmul(psums[m][:, 0:512], lhsT, te_s[:, 0:512],
                                           start=False, stop=True)
                          nc.tensor.matmul(psums[m][:, 512:NCOLS], lhsT, te_s[:, 512:NCOLS],
                                           start=False, stop=True)
                      P = partpool.tile([128, MBL * NCOLS], pdt, tag="part", name="P")
                      for m in range(MBL):
                          nc.scalar.copy(out=P[:, m * NCOLS:(m + 1) * NCOLS], in_=psums[m][:])
                      pin_view = pin[:].rearrange("(m p) n -> p m n", p=128)
                      nc.sync.dma_start(pin_view[:],
                                        P[:].rearrange("p (m n) -> p m n", n=NCOLS))

                  if DEBUG_PARTIALS:
                      for m in range(MBL):
                          nc.sync.dma_start(out_d[128 * m:128 * m + 128, :],
                                            pin[128 * m:128 * m + 128, :])
                  elif "rs" in skip:
                      fin0 = finpool.tile([128, OUTPUT_DIM], f32, tag="fin")
                      z0 = finpool.tile([128, OUTPUT_DIM], pdt, tag="z0")
                      nc.sync.dma_start(z0[:], pin[0:128, 0:OUTPUT_DIM])
                      nc.vector.tensor_copy(out=fin0[:], in_=z0[:])
                      nc.sync.dma_start(out_d[:], fin0[:])
                  else:
                      pout = dpool.tile([128, NCOLS], pdt, tag="pout")
                      nc.gpsimd.collective_compute(
                          "ReduceScatter", mybir.AluOpType.add,
                          replica_groups=rgroups,
                          ins=[pin.opt()], outs=[pout.opt()],
                      )
                      z = finpool.tile([128, NCOLS], pdt, tag="z")
                      nc.sync.dma_start(z[:], pout[:])
                      E = finpool.tile([128, INDIRECT_DIM], f32, tag="E")
                      nc.scalar.activation(E[:], z[:, 0:INDIRECT_DIM],
                                           mybir.ActivationFunctionType.Exp)
                      Em = finpool.tile([128, INDIRECT_DIM], f32, tag="Em")
                      nc.vector.tensor_tensor(out=Em[:], in0=E[:], in1=wpat_s[:],
                                              op=mybir.AluOpType.mult)
                      red = finpool.tile([128, OUTPUT_DIM], f32, tag="red")
                      nc.vector.tensor_reduce(
                          out=red[:], in_=Em[:].rearrange("p (a b) -> p a b", b=2 * NUM_EXPS),
                          axis=mybir.AxisListType.X, op=mybir.AluOpType.add)
                      fin = finpool.tile([128, OUTPUT_DIM], f32, tag="fin")
                      nc.vector.tensor_tensor(out=fin[:], in0=red[:], in1=z[:, INDIRECT_DIM:NCOLS],
                                              op=mybir.AluOpType.add)
                      nc.sync.dma_start(out_d[:], fin[:])

    nc.compile()
    if not skip and not loop_reps and reps == 1:
        _NC = nc
    return nc


def _shard_inputs(x, direct_table, indirect_table):
    comb = np.concatenate([indirect_table, direct_table], axis=1)  # [33152, 576]
    bias = _bias_consts()
    wpat = _wpat_const()
    in_maps = []
    for c in range(NCORES):
        bg, dg = c // DGROUPS, c % DGROUPS
        dims = range(DPC * dg, DPC * (dg + 1))
        bsl = slice(BL * bg, BL * (bg + 1))
        xt = np.ascontiguousarray(x[bsl, DPC * dg:DPC * (dg + 1)].T)  # [32, 512]
        xe = np.repeat(xt, EDGE_ROWS, axis=0)  # [96, 512]
        tmain = np.concatenate(
            [comb[d * DENSITY:d * DENSITY + MAIN_ROWS] for d in dims], axis=0)
        tmain = tmain.reshape(2 * DPC, 128, NCOLS).transpose(1, 0, 2).reshape(128, 2 * DPC * NCOLS)
        tedge = np.concatenate(
            [comb[d * DENSITY + MAIN_ROWS:(d + 1) * DENSITY] for d in dims], axis=0)
        in_maps.append({
            "xt": np.ascontiguousarray(xt, np.float32),
            "xe": np.ascontiguousarray(xe, np.float32),
            "tmain": np.ascontiguousarray(tmain, np.float32),
            "tedge": np.ascontiguousarray(tedge, np.float32),
            "bias": bias,
            "wpat": wpat,
        })
    return in_maps


def kernel(x, direct_table, indirect_table):
    from concourse.bass_utils import run_bass_kernel_spmd
    x = np.asarray(x, np.float32)
    direct_table = np.asarray(direct_table, np.float32)
    indirect_table = np.asarray(indirect_table, np.float32)
    assert x.shape == (BATCH, INPUT_DIM)
    nc = _build()
    in_maps = _shard_inputs(x, direct_table, indirect_table)
    res = run_bass_kernel_spmd(nc, in_maps, core_ids=list(range(NCORES)))
    if DEBUG_PARTIALS:
        return [r["out"] for r in res.results]
    return np.concatenate([r["out"] for r in res.results], axis=0)



# revision 23
# speedup vs baseline: 1.1823x; 1.1823x over previous
# Bass/Trainium2 kernel for nn_ABELSpline (embedding_lookup via implicit one-hot matmul).
#
# Math: out[b,o] = sum_d sum_r B(256*x[b,d] + 3 - r) * table[d*259 + r, o]
#   where B is the cubic B-spline basis on [0,4). The gather/multiply/reduce of the
#   reference is exactly a dense matmul with W^T[r, b] = B(...) built on the fly:
#     B(y) = relu(2-u)^3/6 - (2/3)*relu(1-u)^3,  u = |y-2|
#   TensorE contracts W^T against bf16 tables with PSUM accumulation.
#
# Engine split (per core): TensorE is the ~65us bottleneck; the W build is
# spread so no other engine exceeds it:
#   - groups 0-6 (28 dims): two fused custom DVE ops (7/8 ALU stages) -> ~61us DVE
#   - group 7 (4 dims): ScalarE activation chain (Abs/Relu/Square with per-
#     partition bias) + Pool tensor_tensor combines -> ~19us ScalarE, ~25us Pool
# Tables are pre-converted to bf16 on host (halves table DMA vs f32-cast DMA).
#
# Sharding: cores form a 2x4 grid: batch halves (512) x dim quarters (32 dims).
# A 4-core bf16 ReduceScatter per batch half sums partials, chunked over 4
# batch sub-blocks of 128 so transfer overlaps the tail; each core applies the
# anti-symmetric-exponential tail on its 4x32-batch slices (exp on ScalarE,
# weighted reduce on Pool); the host reassembles the 8 cores' slices.

import numpy as np

BATCH = 1024
INPUT_DIM = 128
DENSITY = 259
OUTPUT_DIM = 64
NUM_EXPS = 4
INDIRECT_DIM = 2 * NUM_EXPS * OUTPUT_DIM  # 512
NCOLS = INDIRECT_DIM + OUTPUT_DIM  # 576
NCORES = 8
DGROUPS = 4                 # dim-parallel cores per batch half
BGROUPS = NCORES // DGROUPS  # 2 batch halves
DPC = INPUT_DIM // DGROUPS  # dims per core = 32
BL = BATCH // BGROUPS       # local batch = 512
MBL = BL // 128             # local batch chunks = 4
GDIMS = 4                   # dims per W build group
NGROUPS = DPC // GDIMS      # 8
SPG = NGROUPS - 1           # the group built on ScalarE+Pool instead of DVE
MAIN_ROWS = 256
EDGE_ROWS = DENSITY - MAIN_ROWS  # 3
EP = DPC * EDGE_ROWS        # 96 edge partitions
SCALE = float(DENSITY - 3)  # 256
K1 = float((1.0 / 6.0) ** (1.0 / 3.0))
K2 = float((4.0 / 6.0) ** (1.0 / 3.0))
RSCH = BL // MBL // DGROUPS  # 32: rows per core per RS chunk

_OPS = None
_NC = None


def _register_ops():
    global _OPS
    if _OPS is not None:
        return _OPS
    from concourse.dve_spec import Spec, Src0, Src1, C0, C1, C2, Bin, relu, sq, minn, lower
    from concourse.dve_uop import AluOp, DveOpSpec
    from concourse import dve_ops
    from concourse.dve_ops import DveOp

    def relu3_body():
        # relu(min(C0 - m, m + C1))^3 with m = Src0*C2
        m = Src0 * C2
        a = Bin(AluOp.SUBTRACT, C0, m)
        b = Bin(AluOp.ADD, m, C1)
        r = relu(minn(a, b))
        return r * sq(r)

    def ref_a(in0, in1, s0, s1, imm2):
        t = np.maximum(np.minimum(s0 - in0 * imm2, in0 * imm2 + s1), 0.0)
        return (t ** 3).astype(np.float32)

    def ref_b(in0, in1, s0, s1, imm2):
        t = np.maximum(np.minimum(s0 - in0 * imm2, in0 * imm2 + s1), 0.0)
        return (in1 - t ** 3).astype(np.float32)

    spec_a = Spec(body=relu3_body(), reference=ref_a)
    spec_b = Spec(body=Bin(AluOp.SUBTRACT, Src1, relu3_body()), reference=ref_b)

    ops = []
    for name, spec in (("SPLINE3A_ANT", spec_a), ("SPLINE3B_ANT", spec_b)):
        if name in dve_ops._SUB_OPCODE_FOR_NAME:
            ops.append(next(o for o in dve_ops.OPS if o.name == name))
            continue
        row = max(dve_ops._SUB_OPCODE_FOR_NAME.values()) + 1
        assert row < 0x20
        uops = lower(spec, ver="v3")
        rd1 = name.endswith("B_ANT")
        sha = DveOpSpec(name=name, opcode=row, uops=uops, rd1_en=rd1).sha("v3")
        op = DveOp(name, spec, subdim=False, uops_sha={"v3": sha})
        dve_ops.OPS.append(op)
        dve_ops._SUB_OPCODE_FOR_NAME[name] = row
        dve_ops.CUSTOM_DVE_SPECS[name] = spec
        ops.append(op)
    _OPS = tuple(ops)
    return _OPS


def _bias_consts():
    # [128, 16] f32 per-partition scalars.
    # cols 4h+{0..3}: main half h (r = 128h + p): K1*(r+1), K1*(3-r), K2*r, K2*(2-r)
    # cols 8..11: edge (r = 256 + p%3)
    # cols 12/13: SP-pipe Abs bias 1-r for half h (arg = 256x - (r-1))
    b = np.zeros((128, 16), np.float32)
    p = np.arange(128, dtype=np.float64)
    for h in (0, 1):
        r = 128 * h + p
        b[:, 4 * h + 0] = K1 * (r + 1)
        b[:, 4 * h + 1] = K1 * (3 - r)
        b[:, 4 * h + 2] = K2 * r
        b[:, 4 * h + 3] = K2 * (2 - r)
        b[:, 12 + h] = 1.0 - r
    b[:, 14] = 2.0 * K1
    b[:, 15] = K2
    re = 256 + (p % 3)
    b[:, 8] = K1 * (re + 1)
    b[:, 9] = K1 * (3 - re)
    b[:, 10] = K2 * re
    b[:, 11] = K2 * (2 - re)
    return b


def _wpat_const():
    w = np.array([1.0 / (i + 1) ** 2 for i in range(NUM_EXPS)], np.float32)
    row = np.concatenate([w, -w])  # [8]
    return np.tile(row, (128, OUTPUT_DIM)).astype(np.float32)  # [128, 512]


def _build(skip=(), loop_reps=0, reps=1):
    global _NC
    if _NC is not None and not skip and not loop_reps and reps == 1:
        return _NC
    import contextlib
    import concourse.bass as bass
    import concourse.bacc as bacc
    import concourse.tile as tile
    import concourse.mybir as mybir

    OP_A, OP_B = _register_ops()
    f32 = mybir.dt.float32
    bf16 = mybir.dt.bfloat16
    AF = mybir.ActivationFunctionType

    nc = bacc.Bacc("TRN2", target_bir_lowering=False, debug=False, num_devices=NCORES)

    xt_d = nc.dram_tensor("xt", [DPC, BL], f32, kind="ExternalInput")
    xe_d = nc.dram_tensor("xe", [EP, BL], f32, kind="ExternalInput")
    # tmain pre-permuted on host (bf16): [128, (2*DPC)*NCOLS], chunk c=2*di+h at cols c*NCOLS
    tmain_d = nc.dram_tensor("tmain", [128, 2 * DPC * NCOLS], bf16, kind="ExternalInput")
    tedge_d = nc.dram_tensor("tedge", [EP, NCOLS], bf16, kind="ExternalInput")
    bias_d = nc.dram_tensor("bias", [128, 16], f32, kind="ExternalInput")
    wpat_d = nc.dram_tensor("wpat", [128, INDIRECT_DIM], f32, kind="ExternalInput")
    out_d = nc.dram_tensor("out", [128, OUTPUT_DIM], f32, kind="ExternalOutput")

    rgroups = [[g * DGROUPS + i for i in range(DGROUPS)] for g in range(BGROUPS)]
    GCH = 2 * GDIMS * NCOLS  # table cols per group chunk-set

    with tile.TileContext(nc) as tc:
        with (
            tc.tile_pool(name="const", bufs=1) as cpool,
            tc.tile_pool(name="tbl", bufs=3) as tpool,
            tc.tile_pool(name="tble", bufs=1) as tepool,
            tc.tile_pool(name="xin", bufs=1) as xpool,
            tc.tile_pool(name="xrep", bufs=5) as xrpool,
            tc.tile_pool(name="w1", bufs=2) as w1pool,
            tc.tile_pool(name="W", bufs=12) as wpool,
            tc.tile_pool(name="We", bufs=1) as wepool,
            tc.tile_pool(name="sp", bufs=5) as spool,
            tc.tile_pool(name="acc", bufs=4, space="PSUM") as psumpool,
            tc.tile_pool(name="part", bufs=2) as partpool,
            tc.tile_pool(name="fin", bufs=1) as finpool,
            tc.tile_pool(name="dram", bufs=2, space="DRAM") as dpool,
        ):
            # ---- constants (outside timing loop) ----
            bias_s = cpool.tile([128, 16], f32, tag="bias")
            nc.sync.dma_start(bias_s[:], bias_d[:])
            wpat_s = cpool.tile([128, INDIRECT_DIM], f32, tag="wpat")
            nc.sync.dma_start(wpat_s[:], wpat_d[:])

            loop_cm = tc.For_i(0, loop_reps, 1) if loop_reps else contextlib.nullcontext()
            with loop_cm:
              for _rep in range(reps):
                # Table quarters stream on the DMA queues in PE-consumption
                # order; x replication to 128 partitions runs on the Pool
                # engine (partition_broadcast) to keep the DMA engines free.
                # x broadcasts (DRAM, partition-stride-0) interleaved with
                # table quarters across both HWDGE queues: DVE's and PE's
                # first inputs arrive first, later groups stream ahead of use.
                xrs, tqs = {}, {}

                def bcast(g, eng):
                    xr = xrpool.tile([128, GDIMS * BL], f32, tag="xrep", name=f"xr{g}")
                    src = bass.AP(xt_d, 0, [[0, 128], [1, GDIMS * BL]])
                    src.offset = g * GDIMS * BL
                    eng.dma_start(xr[:], src)
                    xrs[g] = xr

                def tqload(g, eng, lo=0):
                    tq = tqs.get(g)
                    if tq is None:
                        tq = tpool.tile([128, GCH], bf16, tag="tbl", name=f"tq{g}")
                        tqs[g] = tq
                    eng.dma_start(tq[:, lo:], tmain_d[:, g * GCH + lo:(g + 1) * GCH])
                    return tq

                tq0a = tpool.tile([128, 2 * NCOLS], bf16, tag="tbl0", name="tq0a")
                nc.scalar.dma_start(tq0a[:], tmain_d[:, 0:2 * NCOLS])
                bcast(0, nc.sync)
                bcast(SPG, nc.scalar)
                tqload(0, nc.sync, lo=2 * NCOLS)
                bcast(1, nc.scalar)
                tqload(1, nc.sync)
                bcast(2, nc.scalar)
                xe_s = xpool.tile([EP, BL], f32, tag="xe")
                nc.sync.dma_start(xe_s[:], xe_d[:])
                tqload(2, nc.scalar)
                bcast(3, nc.sync)
                tqload(3, nc.scalar)
                bcast(4, nc.sync)
                tqload(4, nc.scalar)
                bcast(5, nc.sync)
                tqload(5, nc.scalar)
                bcast(6, nc.sync)
                te_s = tepool.tile([EP, NCOLS], bf16, tag="Te")
                nc.scalar.dma_start(te_s[:], tedge_d[:])
                tqload(SPG, nc.sync)
                tqload(SPG - 1, nc.scalar)

                # ---- SP-pipe: W for (7,0),(7,1),(6,1) on ScalarE + Pool ----
                # v = |256x - (r-1)|; W = (K1*relu(2-v))^3 - (K2*relu(1-v))^3
                SP_PAIRS = [(SPG, 0), (SPG, 1)]
                Wsp = {}
                for g, h in SP_PAIRS:
                    d7 = spool.tile([128, GDIMS * BL], f32, tag="sp", name=f"d{g}_{h}")
                    nc.scalar.activation(d7[:], xrs[g][:], AF.Abs,
                                         bias=bias_s[:, 12 + h:13 + h], scale=SCALE)
                    r1 = spool.tile([128, GDIMS * BL], f32, tag="sp", name=f"r1_{g}_{h}")
                    nc.scalar.activation(r1[:], d7[:], AF.Relu,
                                         bias=bias_s[:, 14:15], scale=-K1)
                    s1 = spool.tile([128, GDIMS * BL], f32, tag="sp", name=f"s1_{g}_{h}")
                    nc.scalar.activation(s1[:], r1[:], AF.Square)
                    r2 = spool.tile([128, GDIMS * BL], f32, tag="sp", name=f"r2_{g}_{h}")
                    nc.scalar.activation(r2[:], d7[:], AF.Relu,
                                         bias=bias_s[:, 15:16], scale=-K2)
                    s2 = spool.tile([128, GDIMS * BL], f32, tag="sp", name=f"s2_{g}_{h}")
                    nc.scalar.activation(s2[:], r2[:], AF.Square)
                    t1 = spool.tile([128, GDIMS * BL], f32, tag="sp", name=f"t1_{g}_{h}")
                    nc.gpsimd.tensor_tensor(out=t1[:], in0=r1[:], in1=s1[:],
                                            op=mybir.AluOpType.mult)
                    t2 = spool.tile([128, GDIMS * BL], f32, tag="sp", name=f"t2_{g}_{h}")
                    nc.gpsimd.tensor_tensor(out=t2[:], in0=r2[:], in1=s2[:],
                                            op=mybir.AluOpType.mult)
                    W7 = wpool.tile([128, GDIMS * BL], bf16, tag="W", name=f"Wsp{g}_{h}")
                    nc.gpsimd.tensor_tensor(out=W7[:], in0=t1[:], in1=t2[:],
                                            op=mybir.AluOpType.subtract)
                    Wsp[(g, h)] = W7

                # ---- DVE W build: group 0 in di-granularity (fast PE start),
                #      groups 1..SPG-1 whole; edge rows after group 1 ----
                Wg = {}

                def build_dve(g, h, lo, sz, wtile, wcol):
                    w1 = w1pool.tile([128, sz], bf16, tag="w1")
                    nc.vector._custom_dve(
                        OP_A, out=w1[:], in0=xrs[g][:, lo:lo + sz],
                        s0=bias_s[:, 4 * h:4 * h + 1], s1=bias_s[:, 4 * h + 1:4 * h + 2],
                        imm2=SCALE * K1)
                    nc.vector._custom_dve(
                        OP_B, out=wtile[:, wcol:wcol + sz], in0=xrs[g][:, lo:lo + sz], in1=w1[:],
                        s0=bias_s[:, 4 * h + 2:4 * h + 3], s1=bias_s[:, 4 * h + 3:4 * h + 4],
                        imm2=SCALE * K2)

                for h in (0, 1):
                    subs = []
                    for di in range(GDIMS):
                        Ws = wpool.tile([128, BL], bf16, tag="Ws", name=f"W0_{h}_{di}")
                        build_dve(0, h, di * BL, BL, Ws, 0)
                        subs.append(Ws)
                    Wg[(0, h)] = subs
                for g in range(1, SPG):
                    for h in (0, 1):
                        if (g, h) in Wsp:
                            continue
                        if g == SPG - 1:
                            subs = []
                            for di in range(GDIMS):
                                Ws = wpool.tile([128, BL], bf16, tag="Ws", name=f"W{g}_{h}_{di}")
                                build_dve(g, h, di * BL, BL, Ws, 0)
                                subs.append(Ws)
                            Wg[(g, h)] = subs
                        else:
                            W = wpool.tile([128, GDIMS * BL], bf16, tag="W", name=f"W{g}_{h}")
                            build_dve(g, h, 0, GDIMS * BL, W, 0)
                            Wg[(g, h)] = W
                    if g == 1:
                        w1e = w1pool.tile([EP, BL], bf16, tag="w1", name="w1e")
                        nc.vector._custom_dve(
                            OP_A, out=w1e[:], in0=xe_s[:],
                            s0=bias_s[:EP, 8:9], s1=bias_s[:EP, 9:10], imm2=SCALE * K1)
                        We = wepool.tile([EP, BL], bf16, tag="We", name="We")
                        nc.vector._custom_dve(
                            OP_B, out=We[:], in0=xe_s[:], in1=w1e[:],
                            s0=bias_s[:EP, 10:11], s1=bias_s[:EP, 11:12], imm2=SCALE * K2)
                Wg.update(Wsp)

                def rhslice(g, h, di):
                    if g == 0 and di == 0:
                        return tq0a[:, h * NCOLS:(h + 1) * NCOLS]
                    return tqs[g][:, (2 * di + h) * NCOLS:(2 * di + h + 1) * NCOLS]

                def wslice(g, h, di, m):
                    W = Wg[(g, h)]
                    if isinstance(W, list):
                        return W[di][:, 128 * m:128 * m + 128]
                    return W[:, di * BL + 128 * m:di * BL + 128 * m + 128]

                # ---- main matmuls: psum[m] accumulates [128 batch, 576] ----
                # PE consumes the SP-built group before the last DVE group so
                # it never waits on the slower of the two producers.
                psums = [psumpool.tile([128, NCOLS], f32, tag="acc", name=f"ps{m}")
                         for m in range(MBL)]
                pe_order = list(range(SPG - 1)) + [SPG, SPG - 1]
                pin = dpool.tile([BL, NCOLS], bf16, tag="pin")
                for gi, g in enumerate(pe_order):
                    last_g = gi == len(pe_order) - 1
                    if last_g:
                        # edge rows accumulate before the final group so each
                        # psum chunk finalizes on the final group's matmul
                        for m in range(MBL):
                            lhsT = We[:, 128 * m:128 * m + 128]
                            nc.tensor.matmul(psums[m][:, 0:512], lhsT, te_s[:, 0:512],
                                             start=False, stop=False)
                            nc.tensor.matmul(psums[m][:, 512:NCOLS], lhsT, te_s[:, 512:NCOLS],
                                             start=False, stop=False)
                    if last_g:
                        # h0 streamed JIT per di sub-op; h1 m-major so each
                        # psum chunk finalizes early and packs while PE runs on
                        for di in range(GDIMS):
                            for m in range(MBL):
                                lhsT = wslice(g, 0, di, m)
                                rhs = rhslice(g, 0, di)
                                nc.tensor.matmul(psums[m][:, 0:512], lhsT, rhs[:, 0:512],
                                                 start=False, stop=False)
                                nc.tensor.matmul(psums[m][:, 512:NCOLS], lhsT, rhs[:, 512:NCOLS],
                                                 start=False, stop=False)
                        for m in range(MBL):
                            for di in range(GDIMS):
                                lhsT = wslice(g, 1, di, m)
                                rhs = rhslice(g, 1, di)
                                sp = di == GDIMS - 1
                                nc.tensor.matmul(psums[m][:, 0:512], lhsT, rhs[:, 0:512],
                                                 start=False, stop=sp)
                                nc.tensor.matmul(psums[m][:, 512:NCOLS], lhsT, rhs[:, 512:NCOLS],
                                                 start=False, stop=sp)
                            P = partpool.tile([128, NCOLS], bf16, tag="part", name=f"P{m}")
                            nc.scalar.copy(out=P[:], in_=psums[m][:])
                            nc.sync.dma_start(pin[128 * m:128 * (m + 1), :], P[:])
                    else:
                        for m in range(MBL):
                            for h in (0, 1):
                                for di in range(GDIMS):
                                    lhsT = wslice(g, h, di, m)
                                    rhs = rhslice(g, h, di)
                                    st = gi == 0 and h == 0 and di == 0
                                    nc.tensor.matmul(psums[m][:, 0:512], lhsT, rhs[:, 0:512],
                                                     start=st, stop=False)
                                    nc.tensor.matmul(psums[m][:, 512:NCOLS], lhsT, rhs[:, 512:NCOLS],
                                                     start=st, stop=False)

                if "rs" in skip:
                    zsrc = pin[0:128, :]
                else:
                    pout = dpool.tile([128, NCOLS], bf16, tag="pout")
                    nc.gpsimd.collective_compute(
                        "ReduceScatter", mybir.AluOpType.add,
                        replica_groups=rgroups,
                        ins=[pin.opt()], outs=[pout.opt()],
                    )
                    zsrc = pout[:]
                zb = finpool.tile([128, NCOLS], bf16, tag="zb")
                nc.sync.dma_start(zb[:], zsrc)
                E = finpool.tile([128, INDIRECT_DIM], f32, tag="E")
                nc.scalar.activation(E[:], zb[:, 0:INDIRECT_DIM], AF.Exp)
                zd = finpool.tile([128, OUTPUT_DIM], f32, tag="zd")
                nc.scalar.copy(out=zd[:], in_=zb[:, INDIRECT_DIM:NCOLS])
                Em = finpool.tile([128, INDIRECT_DIM], f32, tag="Em")
                nc.vector.tensor_tensor(out=Em[:], in0=E[:], in1=wpat_s[:],
                                        op=mybir.AluOpType.mult)
                red = finpool.tile([128, OUTPUT_DIM], f32, tag="red")
                nc.vector.tensor_reduce(
                    out=red[:], in_=Em[:].rearrange("p (a b) -> p a b", b=2 * NUM_EXPS),
                    axis=mybir.AxisListType.X, op=mybir.AluOpType.add)
                fin = finpool.tile([128, OUTPUT_DIM], f32, tag="fin")
                nc.vector.tensor_tensor(out=fin[:], in0=red[:], in1=zd[:],
                                        op=mybir.AluOpType.add)
                nc.sync.dma_start(out_d[:], fin[:])

    nc.compile()
    if not skip and not loop_reps and reps == 1:
        _NC = nc
    return nc


def _shard_inputs(x, direct_table, indirect_table):
    import ml_dtypes
    bf16 = ml_dtypes.bfloat16
    comb = np.concatenate([indirect_table, direct_table], axis=1)  # [33152, 576]
    bias = _bias_consts()
    wpat = _wpat_const()
    in_maps = []
    for c in range(NCORES):
        bg, dg = c // DGROUPS, c % DGROUPS
        dims = range(DPC * dg, DPC * (dg + 1))
        bsl = slice(BL * bg, BL * (bg + 1))
        xt = np.ascontiguousarray(x[bsl, DPC * dg:DPC * (dg + 1)].T)  # [32, 512]
        xe = np.repeat(xt, EDGE_ROWS, axis=0)  # [96, 512]
        tmain = np.concatenate(
            [comb[d * DENSITY:d * DENSITY + MAIN_ROWS] for d in dims], axis=0)
        tmain = tmain.reshape(2 * DPC, 128, NCOLS).transpose(1, 0, 2).reshape(128, 2 * DPC * NCOLS)
        tedge = np.concatenate(
            [comb[d * DENSITY + MAIN_ROWS:(d + 1) * DENSITY] for d in dims], axis=0)
        in_maps.append({
            "xt": np.ascontiguousarray(xt, np.float32),
            "xe": np.ascontiguousarray(xe, np.float32),
            "tmain": np.ascontiguousarray(tmain).astype(bf16),
            "tedge": np.ascontiguousarray(tedge).astype(bf16),
            "bias": bias,
            "wpat": wpat,
        })
    return in_maps


def kernel(x, direct_table, indirect_table):
    from concourse.bass_utils import run_bass_kernel_spmd
    x = np.asarray(x, np.float32)
    direct_table = np.asarray(direct_table, np.float32)
    indirect_table = np.asarray(indirect_table, np.float32)
    assert x.shape == (BATCH, INPUT_DIM)
    nc = _build()
    in_maps = _shard_inputs(x, direct_table, indirect_table)
    res = run_bass_kernel_spmd(nc, in_maps, core_ids=list(range(NCORES)))
    return np.concatenate([r["out"] for r in res.results], axis=0)
